# revision 8
# baseline (speedup 1.0000x reference)
"""Trainium2 Bass kernel for a GNN BasicBlock (sparse conv x2 + BN + residual).

Computes, for full inputs (N=50000 points, C=128 channels, K=27 offsets):
    out = relu(bn2(conv(relu(bn1(conv(x, w1))), w2)) + x)
where conv(x, w)[n] = sum_k x[nbr_idx[n, k]] @ w[k].

Strategy (8 NeuronCores):
  - Shard points (rows of x / nbr_idx) evenly across the 8 cores; replicate
    the feature table and weights.  BN is folded on the host (scale into the
    weights, shift into a per-channel bias applied by the ReLU activation).
  - One Bass program is compiled once and executed twice (layer 1 with
    res=0, layer 2 with res=x); the host reassembles the full feature table
    between executions (the "all-gather").
  - Neighbor gather: GpSimd dma_gather in NATURAL layout (each gathered
    element is a 512B contiguous write into one partition; the transposed
    gather mode writes 2B per partition and is ~2.5x slower on HW).  The
    feature table is stored as fp16 row pairs ([N/2, 2*C]) so elements are
    512B and pair indices fit int16.  Each macro tile's gather is split
    across the 4 SWDGE queues (~2x faster than one queue on HW).
  - Element i of a gather lands at [partition i%128, slot i//128]; the host
    orders indices as i = (kk*mt + h)*128 + q so slot s = (kk, h) holds
    neighbor kk of point h*128+q.  A per-partition parity mask (uint8,
    host-provided) selects the odd row of each pair with a single DVE
    copy_predicated per macro; TensorE transposes each [128pt, 128c] chunk
    into matmul layout, PSUM->SBUF copies alternate between DVE and ACT.
  - 27 fp16 matmuls (weights stationary) accumulate into one PSUM bank per
    macro tile; the residual is added with an identity matmul and a single
    ScalarE activation applies relu(acc + bias) on the way out.
  - Outputs are produced transposed ([C, pts]); the host transposes back.
"""

import dataclasses
import sys

if "/opt/trn_rl_repo" not in sys.path:
    sys.path.insert(0, "/opt/trn_rl_repo")

from contextlib import ExitStack

import numpy as np

import concourse.bass as bass
import concourse.tile as tile
from concourse import bacc, mybir
from concourse.bass_utils import run_bass_kernel_spmd
from concourse.masks import make_identity

F32 = mybir.dt.float32
F16 = mybir.dt.float16
I16 = mybir.dt.int16
U8 = mybir.dt.uint8

N, C, K = 50000, 128, 27
EPS = 1e-5
NCORES = 8
SHARD = N // NCORES          # 6250 points per core
PT = 128                     # points per tile
NT = -(-SHARD // PT)         # 49 point tiles per core
PTS_PAD = NT * PT            # 6272 padded points per core
MACRO_TILES = 2              # point tiles per macro tile (matmul N = 256)
NQ = 4                       # SWDGE queues used for the gather


def _macro_sizes(nt, macro_tiles):
    out = []
    t0 = 0
    while t0 < nt:
        out.append(min(macro_tiles, nt - t0))
        t0 += macro_tiles
    return out


def build_program(n_table=N, n_pts=PTS_PAD, k=K, c=C, macro_tiles=MACRO_TILES,
                  num_devices=NCORES, repeat=1, variant="full"):
    """n_table must be even; the gather table holds n_table//2 row pairs.

    repeat>1 wraps the body in a device-side loop re-running the same work
    (benchmarking only: isolates device time from dispatch overhead).
    variant: "full" | "gather_only" | "no_gather"."""
    nt = n_pts // PT
    sizes = _macro_sizes(nt, macro_tiles)
    n_macros = len(sizes)
    slots_max = macro_tiles * k
    ni_max = slots_max * PT

    nc = bacc.Bacc(
        "TRN2",
        target_bir_lowering=False,
        debug=False,
        enable_asserts=False,
        num_devices=num_devices,
        num_swdge_queues=NQ,
    )
    # +2 pad pair-rows: the gather reads overlapping 4-row windows (1024B
    # elements at 512B stride), so the last pair's window spills one pair over
    xp_dram = nc.dram_tensor("x_pairs", [n_table // 2 + 2, 2 * c], F16,
                             kind="ExternalInput").ap()
    idx_dram = nc.dram_tensor("idx16", [n_macros, PT, ni_max // 16], I16,
                              kind="ExternalInput").ap()
    msk_dram = nc.dram_tensor("mask", [n_macros, PT, slots_max], U8,
                              kind="ExternalInput").ap()
    w_dram = nc.dram_tensor("w", [c, k * c], F16, kind="ExternalInput").ap()
    b_dram = nc.dram_tensor("bias", [c, 1], F32, kind="ExternalInput").ap()
    res_dram = nc.dram_tensor("resT", [c, n_pts], F16, kind="ExternalInput").ap()
    out_dram = nc.dram_tensor("outT", [c, n_pts], F32, kind="ExternalOutput").ap()

    with tile.TileContext(nc) as tc, ExitStack() as ctx:
        const_pool = ctx.enter_context(tc.tile_pool(name="const", bufs=1))
        idx_pool = ctx.enter_context(tc.tile_pool(name="idx", bufs=3))
        msk_pool = ctx.enter_context(tc.tile_pool(name="msk", bufs=3))
        g_pool = ctx.enter_context(tc.tile_pool(name="g", bufs=3))
        rhs_pool = ctx.enter_context(tc.tile_pool(name="rhs", bufs=4))
        res_pool = ctx.enter_context(tc.tile_pool(name="res", bufs=2))
        out_pool = ctx.enter_context(tc.tile_pool(name="out", bufs=2))
        pt_pool = ctx.enter_context(tc.tile_pool(name="pt", bufs=4, space="PSUM"))
        acc_pool = ctx.enter_context(tc.tile_pool(name="acc", bufs=2, space="PSUM"))

        identf = const_pool.tile([PT, PT], F32)
        make_identity(nc, identf[:])
        ident = const_pool.tile([PT, PT], F16)
        nc.vector.tensor_copy(ident[:], identf[:])
        w_sb = const_pool.tile([c, k * c], F16)
        nc.sync.dma_start(w_sb[:], w_dram[:, :])
        bias_sb = const_pool.tile([c, 1], F32)
        nc.sync.dma_start(bias_sb[:], b_dram[:, :])
        # whole-layer residual in / output staging in SBUF: one big DMA each
        # instead of 128 strided descriptors per macro
        res_sb = const_pool.tile([c, n_pts], F16)
        nc.sync.dma_start(res_sb[:], res_dram[:, :])
        out_sb = const_pool.tile([c, n_pts], F32)

        def emit_macro(m, mt, t0):
            npts = mt * PT
            slots = mt * k
            ni = slots * PT
            it = idx_pool.tile([PT, ni // 16], I16, tag="idx")
            nc.sync.dma_start(it[:, :], idx_dram[m][:, : ni // 16])
            pm = msk_pool.tile([PT, slots], U8, tag="msk")
            nc.sync.dma_start(pm[:, :], msk_dram[m][:, :slots])

            # 512B pair elements: pair index j reads rows (2j, 2j+1); the DMA
            # engines are byte-bound (~24ns/512B desc vs ~45.5ns/1024B on HW),
            # so halving the element size halves gather time
            elem = 2 * c
            table = dataclasses.replace(
                xp_dram[:, :], ap=[[2 * c, n_table // 2], [1, elem]]
            )
            g = g_pool.tile([PT, slots, elem], F16, tag="g")
            if variant != "no_gather":
                # chunks of <=7 slots (896 idxs = 57 of the 128 SWDGE ring
                # entries) so two chunks fit per queue ring and the DMA
                # engines never starve between macros
                s0 = 0
                h = 0
                while s0 < slots:
                    s1 = min(s0 + 7, slots)
                    nih = (s1 - s0) * PT
                    nc.gpsimd.dma_gather(
                        out_ap=g[:, s0:s1, :],
                        in_ap=table,
                        idxs_ap=it[:, s0 * PT // 16 : s1 * PT // 16],
                        num_idxs=nih,
                        num_idxs_reg=nih,
                        elem_size=elem,
                        elem_step=2 * c,
                        transpose=False,
                        single_packet=False,
                        queue_num=h % NQ,
                    )
                    s0 = s1
                    h += 1
            else:
                nc.vector.memset(g[:, :, :16], 0.25)
            if variant == "gather_only":
                out_t = out_pool.tile([c, npts], F32)
                nc.vector.tensor_copy(out_t[:, :16], g[:, 0, :16])
                nc.sync.dma_start(
                    out_dram[:, t0 * PT : t0 * PT + 16], out_t[:, :16]
                )
                return

            # select the odd row of each gathered pair where parity==1;
            # f32 bitcast halves the element count (copy_predicated runs at
            # ~1.09ns/elem regardless of dtype)
            g32 = g[:, :, :].bitcast(F32)
            nc.vector.copy_predicated(
                g32[:, :, 0 : c // 2],
                pm[:, :].to_broadcast([PT, slots, c // 2]),
                g32[:, :, c // 2 : c],
            )

            acc = acc_pool.tile([PT, npts], F32, space="PSUM")
            for kk in range(k):
                pt_ps = pt_pool.tile([PT, npts], F16, space="PSUM")
                for h in range(mt):
                    nc.tensor.transpose(
                        pt_ps[:, h * PT : (h + 1) * PT],
                        g[:, kk * mt + h, 0:c],
                        ident[:],
                    )
                rhs_t = rhs_pool.tile([PT, npts], F16, tag="rhs")
                eng = nc.vector if kk % 2 == 0 else nc.scalar
                if eng is nc.vector:
                    eng.tensor_copy(rhs_t[:], pt_ps[:])
                else:
                    eng.copy(rhs_t[:], pt_ps[:])
                nc.tensor.matmul(
                    acc[:],
                    lhsT=w_sb[:, kk * c : (kk + 1) * c],
                    rhs=rhs_t[:],
                    start=(kk == 0),
                    stop=False,
                )
            nc.tensor.matmul(
                acc[:],
                lhsT=ident[:],
                rhs=res_sb[:, t0 * PT : t0 * PT + npts],
                start=False,
                stop=True,
            )
            nc.scalar.activation(
                out_sb[:, t0 * PT : t0 * PT + npts],
                acc[:],
                mybir.ActivationFunctionType.Relu,
                bias=bias_sb[:, :1],
                scale=1.0,
            )

        def emit_all():
            t0 = 0
            for m, mt in enumerate(sizes):
                emit_macro(m, mt, t0)
                t0 += mt
            if variant != "gather_only":
                nc.sync.dma_start(out_dram[:, :], out_sb[:])

        if repeat > 1:
            with tc.For_i(0, repeat, 1):
                emit_all()
        else:
            emit_all()
    nc.compile()
    return nc


_PROGRAM = None


def _get_program():
    global _PROGRAM
    if _PROGRAM is None:
        _PROGRAM = build_program()
    return _PROGRAM


def _fold_bn(w, g, b, m, v):
    s = (g / np.sqrt(v + EPS)).astype(np.float32)
    t = (b - m * s).astype(np.float32)
    wf = (w * s[None, None, :]).transpose(1, 0, 2).reshape(C, K * C)
    return np.ascontiguousarray(wf, np.float16), t.reshape(C, 1).astype(np.float32)


def prep_indices(nbr_idx, n_pts=PTS_PAD, k=K, macro_tiles=MACRO_TILES,
                 shard=None):
    """Per-shard gather indices and parity masks.

    Returns idx16 [n_macros, 128, ni_max/16] int16 (16-wrapped + replicated
    pair indices, flat order i = (kk*mt+h)*128 + q) and parity mask
    [n_macros, 128, slots_max] uint8 (mask[q, kk*mt+h] = nbr[h*128+q, kk]&1).
    """
    nt = n_pts // PT
    sizes = _macro_sizes(nt, macro_tiles)
    n_macros = len(sizes)
    slots_max = macro_tiles * k
    ni_max = slots_max * PT
    idx16 = np.zeros((n_macros, PT, ni_max // 16), np.int16)
    mask = np.zeros((n_macros, PT, slots_max), np.uint8)
    rows = nbr_idx if shard is None else nbr_idx[shard[0] : shard[1]]
    if rows.shape[0] < n_pts:
        pad = np.zeros((n_pts - rows.shape[0], k), rows.dtype)
        rows = np.concatenate([rows, pad], axis=0)
    t0 = 0
    for m, mt in enumerate(sizes):
        npts = mt * PT
        slots = mt * k
        ni = slots * PT
        blk = rows[t0 * PT : t0 * PT + npts]            # [npts, k]
        # flat[(kk*mt+h)*128+q] = blk[h*128+q, kk]
        flat = blk.reshape(mt, PT, k).transpose(2, 0, 1).reshape(ni)
        pair = (flat >> 1).astype(np.int16)
        mask[m, :, :slots] = (flat & 1).reshape(slots, PT).T
        wrapped = pair.reshape(ni // 16, 16).T          # [16, ni/16]
        idx16[m, :, : ni // 16] = np.tile(wrapped, (PT // 16, 1))
        t0 += mt
    return idx16, mask


TRACE = False
LAST_EXEC_NS = []


def _run_layer(nc, xp, idx_shards, msk_shards, wf, t, res_shards):
    in_maps = []
    for ci in range(NCORES):
        in_maps.append(
            {
                "x_pairs": xp,
                "idx16": idx_shards[ci],
                "mask": msk_shards[ci],
                "w": wf,
                "bias": t,
                "resT": res_shards[ci],
            }
        )
    r = run_bass_kernel_spmd(nc, in_maps, core_ids=list(range(NCORES)),
                             trace=TRACE)
    if TRACE:
        LAST_EXEC_NS.append(
            (r.exec_time_ns, r.mean_exec_time_ns, r.instructions_and_trace)
        )
    outs = [r.results[ci]["outT"][:, :SHARD].T for ci in range(NCORES)]
    return np.ascontiguousarray(np.concatenate(outs, axis=0), np.float32)


def _to_pairs(x):
    xp = np.zeros((N // 2 + 2, 2 * C), np.float16)
    xp[: N // 2] = x.astype(np.float16).reshape(N // 2, 2 * C)
    return xp


def kernel(x, w1, g1, b1, m1, v1, w2, g2, b2, m2, v2, nbr_idx):
    x = np.ascontiguousarray(x, np.float32)
    nbr_idx = np.ascontiguousarray(nbr_idx, np.int32)
    w1f, t1 = _fold_bn(np.asarray(w1, np.float32), g1, b1, m1, v1)
    w2f, t2 = _fold_bn(np.asarray(w2, np.float32), g2, b2, m2, v2)

    nc = _get_program()
    idx_shards, msk_shards = [], []
    for ci in range(NCORES):
        i16, mk = prep_indices(nbr_idx, shard=(ci * SHARD, (ci + 1) * SHARD))
        idx_shards.append(i16)
        msk_shards.append(mk)

    zero_res = np.zeros((C, PTS_PAD), np.float16)
    out1 = _run_layer(nc, _to_pairs(x), idx_shards, msk_shards, w1f, t1,
                      [zero_res] * NCORES)

    res_shards = []
    for ci in range(NCORES):
        sh = np.zeros((C, PTS_PAD), np.float16)
        sh[:, :SHARD] = x[ci * SHARD : (ci + 1) * SHARD].astype(np.float16).T
        res_shards.append(sh)
    out2 = _run_layer(nc, _to_pairs(out1), idx_shards, msk_shards, w2f, t2,
                      res_shards)
    return out2



# revision 9
# speedup vs baseline: 1.0719x; 1.0719x over previous
"""Trainium2 Bass kernel for a GNN BasicBlock (sparse conv x2 + BN + residual).

Computes, for full inputs (N=50000 points, C=128 channels, K=27 offsets):
    out = relu(bn2(conv(relu(bn1(conv(x, w1))), w2)) + x)
where conv(x, w)[n] = sum_k x[nbr_idx[n, k]] @ w[k].

Strategy (8 NeuronCores):
  - Shard points (rows of x / nbr_idx) evenly across the 8 cores; replicate
    the feature table and weights.  BN is folded on the host (scale into the
    weights, shift into a per-channel bias applied by the ReLU activation).
  - One Bass program is compiled once and executed twice (layer 1 with
    res=0, layer 2 with res=x); the host reassembles the full feature table
    between executions (the "all-gather").
  - Neighbor gather: GpSimd dma_gather in NATURAL layout (each gathered
    element is a 512B contiguous write into one partition; the transposed
    gather mode writes 2B per partition and is ~2.5x slower on HW).  The
    feature table is stored as fp16 row pairs ([N/2, 2*C]) so elements are
    512B and pair indices fit int16.  Each macro tile's gather is split
    across the 4 SWDGE queues (~2x faster than one queue on HW).
  - Element i of a gather lands at [partition i%128, slot i//128]; the host
    orders indices as i = (kk*mt + h)*128 + q so slot s = (kk, h) holds
    neighbor kk of point h*128+q.  A per-partition parity mask (uint8,
    host-provided) selects the odd row of each pair with a single DVE
    copy_predicated per macro; TensorE transposes each [128pt, 128c] chunk
    into matmul layout, PSUM->SBUF copies alternate between DVE and ACT.
  - 27 fp16 matmuls (weights stationary) accumulate into one PSUM bank per
    macro tile; the residual is added with an identity matmul and a single
    ScalarE activation applies relu(acc + bias) on the way out.
  - Outputs are produced transposed ([C, pts]); the host transposes back.
"""

import dataclasses
import sys

if "/opt/trn_rl_repo" not in sys.path:
    sys.path.insert(0, "/opt/trn_rl_repo")

from contextlib import ExitStack

import numpy as np

import concourse.bass as bass
import concourse.tile as tile
from concourse import bacc, mybir
from concourse.bass_utils import run_bass_kernel_spmd
from concourse.masks import make_identity

F32 = mybir.dt.float32
F16 = mybir.dt.float16
I16 = mybir.dt.int16
U8 = mybir.dt.uint8

N, C, K = 50000, 128, 27
EPS = 1e-5
NCORES = 8
SHARD = N // NCORES          # 6250 points per core
PT = 128                     # points per tile
NT = -(-SHARD // PT)         # 49 point tiles per core
PTS_PAD = NT * PT            # 6272 padded points per core
MACRO_TILES = 2              # point tiles per macro tile (matmul N = 256)
NQ = 4                       # SWDGE queues used for the gather


def _macro_sizes(nt, macro_tiles):
    out = []
    t0 = 0
    while t0 < nt:
        out.append(min(macro_tiles, nt - t0))
        t0 += macro_tiles
    return out


def build_program(n_table=N, n_pts=PTS_PAD, k=K, c=C, macro_tiles=MACRO_TILES,
                  num_devices=NCORES, repeat=1, variant="full"):
    """n_table must be even; the gather table holds n_table//2 row pairs.

    repeat>1 wraps the body in a device-side loop re-running the same work
    (benchmarking only: isolates device time from dispatch overhead).
    variant: "full" | "gather_only" | "no_gather"."""
    nt = n_pts // PT
    sizes = _macro_sizes(nt, macro_tiles)
    n_macros = len(sizes)
    slots_max = macro_tiles * k
    ni_max = slots_max * PT

    nc = bacc.Bacc(
        "TRN2",
        target_bir_lowering=False,
        debug=False,
        enable_asserts=False,
        num_devices=num_devices,
        num_swdge_queues=NQ,
    )
    # +2 pad pair-rows: the gather reads overlapping 4-row windows (1024B
    # elements at 512B stride), so the last pair's window spills one pair over
    xp_dram = nc.dram_tensor("x_pairs", [n_table // 2 + 2, 2 * c], F16,
                             kind="ExternalInput").ap()
    idx_dram = nc.dram_tensor("idx16", [n_macros, PT, ni_max // 16], I16,
                              kind="ExternalInput").ap()
    msk_dram = nc.dram_tensor("mask", [n_macros, PT, slots_max], U8,
                              kind="ExternalInput").ap()
    w_dram = nc.dram_tensor("w", [c, k * c], F16, kind="ExternalInput").ap()
    b_dram = nc.dram_tensor("bias", [c, 1], F32, kind="ExternalInput").ap()
    res_dram = nc.dram_tensor("resT", [c, n_pts], F16, kind="ExternalInput").ap()
    out_dram = nc.dram_tensor("outT", [c, n_pts], F32, kind="ExternalOutput").ap()

    with tile.TileContext(nc) as tc, ExitStack() as ctx:
        const_pool = ctx.enter_context(tc.tile_pool(name="const", bufs=1))
        idx_pool = ctx.enter_context(tc.tile_pool(name="idx", bufs=3))
        msk_pool = ctx.enter_context(tc.tile_pool(name="msk", bufs=3))
        g_pool = ctx.enter_context(tc.tile_pool(name="g", bufs=3))
        rhs_pool = ctx.enter_context(tc.tile_pool(name="rhs", bufs=4))
        res_pool = ctx.enter_context(tc.tile_pool(name="res", bufs=2))
        out_pool = ctx.enter_context(tc.tile_pool(name="out", bufs=2))
        pt_pool = ctx.enter_context(tc.tile_pool(name="pt", bufs=4, space="PSUM"))
        acc_pool = ctx.enter_context(tc.tile_pool(name="acc", bufs=2, space="PSUM"))

        identf = const_pool.tile([PT, PT], F32)
        make_identity(nc, identf[:])
        ident = const_pool.tile([PT, PT], F16)
        nc.vector.tensor_copy(ident[:], identf[:])
        w_sb = const_pool.tile([c, k * c], F16)
        nc.sync.dma_start(w_sb[:], w_dram[:, :])
        bias_sb = const_pool.tile([c, 1], F32)
        nc.sync.dma_start(bias_sb[:], b_dram[:, :])
        # whole-layer residual in / output staging in SBUF: one big DMA each
        # instead of 128 strided descriptors per macro
        res_sb = const_pool.tile([c, n_pts], F16)
        nc.sync.dma_start(res_sb[:], res_dram[:, :])
        out_sb = const_pool.tile([c, n_pts], F32)

        def emit_macro(m, mt, t0):
            npts = mt * PT
            slots = mt * k
            ni = slots * PT
            it = idx_pool.tile([PT, ni // 16], I16, tag="idx")
            nc.sync.dma_start(it[:, :], idx_dram[m][:, : ni // 16])
            pm = msk_pool.tile([PT, slots], U8, tag="msk")
            nc.sync.dma_start(pm[:, :], msk_dram[m][:, :slots])

            # 512B pair elements: pair index j reads rows (2j, 2j+1); the DMA
            # engines are byte-bound (~24ns/512B desc vs ~45.5ns/1024B on HW),
            # so halving the element size halves gather time
            elem = 2 * c
            table = dataclasses.replace(
                xp_dram[:, :], ap=[[2 * c, n_table // 2], [1, elem]]
            )
            g = g_pool.tile([PT, slots, elem], F16, tag="g")
            if variant != "no_gather":
                bounds = [round(i * slots / NQ) for i in range(NQ + 1)]
                for h in range(NQ):
                    s0, s1 = bounds[h], bounds[h + 1]
                    if s1 <= s0:
                        continue
                    nih = (s1 - s0) * PT
                    nc.gpsimd.dma_gather(
                        out_ap=g[:, s0:s1, :],
                        in_ap=table,
                        idxs_ap=it[:, s0 * PT // 16 : s1 * PT // 16],
                        num_idxs=nih,
                        num_idxs_reg=nih,
                        elem_size=elem,
                        elem_step=2 * c,
                        transpose=False,
                        single_packet=False,
                        queue_num=h,
                    )
            else:
                nc.vector.memset(g[:, :, :16], 0.25)
            if variant == "gather_only":
                out_t = out_pool.tile([c, npts], F32)
                nc.vector.tensor_copy(out_t[:, :16], g[:, 0, :16])
                nc.sync.dma_start(
                    out_dram[:, t0 * PT : t0 * PT + 16], out_t[:, :16]
                )
                return

            # select the odd row of each gathered pair where parity==1;
            # f32 bitcast halves the element count (copy_predicated runs at
            # ~1.09ns/elem regardless of dtype)
            g32 = g[:, :, :].bitcast(F32)
            nc.vector.copy_predicated(
                g32[:, :, 0 : c // 2],
                pm[:, :].to_broadcast([PT, slots, c // 2]),
                g32[:, :, c // 2 : c],
            )

            acc = acc_pool.tile([PT, npts], F32, space="PSUM")
            for kk in range(k):
                pt_ps = pt_pool.tile([PT, npts], F16, space="PSUM")
                for h in range(mt):
                    nc.tensor.transpose(
                        pt_ps[:, h * PT : (h + 1) * PT],
                        g[:, kk * mt + h, 0:c],
                        ident[:],
                    )
                rhs_t = rhs_pool.tile([PT, npts], F16, tag="rhs")
                eng = nc.vector if kk % 2 == 0 else nc.scalar
                if eng is nc.vector:
                    eng.tensor_copy(rhs_t[:], pt_ps[:])
                else:
                    eng.copy(rhs_t[:], pt_ps[:])
                nc.tensor.matmul(
                    acc[:],
                    lhsT=w_sb[:, kk * c : (kk + 1) * c],
                    rhs=rhs_t[:],
                    start=(kk == 0),
                    stop=False,
                )
            nc.tensor.matmul(
                acc[:],
                lhsT=ident[:],
                rhs=res_sb[:, t0 * PT : t0 * PT + npts],
                start=False,
                stop=True,
            )
            nc.scalar.activation(
                out_sb[:, t0 * PT : t0 * PT + npts],
                acc[:],
                mybir.ActivationFunctionType.Relu,
                bias=bias_sb[:, :1],
                scale=1.0,
            )

        def emit_all():
            t0 = 0
            for m, mt in enumerate(sizes):
                emit_macro(m, mt, t0)
                t0 += mt
            if variant != "gather_only":
                nc.sync.dma_start(out_dram[:, :], out_sb[:])

        if repeat > 1:
            with tc.For_i(0, repeat, 1):
                emit_all()
        else:
            emit_all()
    nc.compile()
    return nc


_PROGRAM = None


def _get_program():
    global _PROGRAM
    if _PROGRAM is None:
        _PROGRAM = build_program()
    return _PROGRAM


def _fold_bn(w, g, b, m, v):
    s = (g / np.sqrt(v + EPS)).astype(np.float32)
    t = (b - m * s).astype(np.float32)
    wf = (w * s[None, None, :]).transpose(1, 0, 2).reshape(C, K * C)
    return np.ascontiguousarray(wf, np.float16), t.reshape(C, 1).astype(np.float32)


def prep_indices(nbr_idx, n_pts=PTS_PAD, k=K, macro_tiles=MACRO_TILES,
                 shard=None):
    """Per-shard gather indices and parity masks.

    Returns idx16 [n_macros, 128, ni_max/16] int16 (16-wrapped + replicated
    pair indices, flat order i = (kk*mt+h)*128 + q) and parity mask
    [n_macros, 128, slots_max] uint8 (mask[q, kk*mt+h] = nbr[h*128+q, kk]&1).
    """
    nt = n_pts // PT
    sizes = _macro_sizes(nt, macro_tiles)
    n_macros = len(sizes)
    slots_max = macro_tiles * k
    ni_max = slots_max * PT
    idx16 = np.zeros((n_macros, PT, ni_max // 16), np.int16)
    mask = np.zeros((n_macros, PT, slots_max), np.uint8)
    rows = nbr_idx if shard is None else nbr_idx[shard[0] : shard[1]]
    if rows.shape[0] < n_pts:
        pad = np.zeros((n_pts - rows.shape[0], k), rows.dtype)
        rows = np.concatenate([rows, pad], axis=0)
    t0 = 0
    for m, mt in enumerate(sizes):
        npts = mt * PT
        slots = mt * k
        ni = slots * PT
        blk = rows[t0 * PT : t0 * PT + npts]            # [npts, k]
        # flat[(kk*mt+h)*128+q] = blk[h*128+q, kk]
        flat = blk.reshape(mt, PT, k).transpose(2, 0, 1).reshape(ni)
        pair = (flat >> 1).astype(np.int16)
        mask[m, :, :slots] = (flat & 1).reshape(slots, PT).T
        wrapped = pair.reshape(ni // 16, 16).T          # [16, ni/16]
        idx16[m, :, : ni // 16] = np.tile(wrapped, (PT // 16, 1))
        t0 += mt
    return idx16, mask


TRACE = False
LAST_EXEC_NS = []


def _run_layer(nc, xp, idx_shards, msk_shards, wf, t, res_shards):
    in_maps = []
    for ci in range(NCORES):
        in_maps.append(
            {
                "x_pairs": xp,
                "idx16": idx_shards[ci],
                "mask": msk_shards[ci],
                "w": wf,
                "bias": t,
                "resT": res_shards[ci],
            }
        )
    r = run_bass_kernel_spmd(nc, in_maps, core_ids=list(range(NCORES)),
                             trace=TRACE)
    if TRACE:
        LAST_EXEC_NS.append(
            (r.exec_time_ns, r.mean_exec_time_ns, r.instructions_and_trace)
        )
    outs = [r.results[ci]["outT"][:, :SHARD].T for ci in range(NCORES)]
    return np.ascontiguousarray(np.concatenate(outs, axis=0), np.float32)


def _to_pairs(x):
    xp = np.zeros((N // 2 + 2, 2 * C), np.float16)
    xp[: N // 2] = x.astype(np.float16).reshape(N // 2, 2 * C)
    return xp


def kernel(x, w1, g1, b1, m1, v1, w2, g2, b2, m2, v2, nbr_idx):
    x = np.ascontiguousarray(x, np.float32)
    nbr_idx = np.ascontiguousarray(nbr_idx, np.int32)
    w1f, t1 = _fold_bn(np.asarray(w1, np.float32), g1, b1, m1, v1)
    w2f, t2 = _fold_bn(np.asarray(w2, np.float32), g2, b2, m2, v2)

    nc = _get_program()
    idx_shards, msk_shards = [], []
    for ci in range(NCORES):
        i16, mk = prep_indices(nbr_idx, shard=(ci * SHARD, (ci + 1) * SHARD))
        idx_shards.append(i16)
        msk_shards.append(mk)

    zero_res = np.zeros((C, PTS_PAD), np.float16)
    out1 = _run_layer(nc, _to_pairs(x), idx_shards, msk_shards, w1f, t1,
                      [zero_res] * NCORES)

    res_shards = []
    for ci in range(NCORES):
        sh = np.zeros((C, PTS_PAD), np.float16)
        sh[:, :SHARD] = x[ci * SHARD : (ci + 1) * SHARD].astype(np.float16).T
        res_shards.append(sh)
    out2 = _run_layer(nc, _to_pairs(out1), idx_shards, msk_shards, w2f, t2,
                      res_shards)
    return out2

